# revision 44
# baseline (speedup 1.0000x reference)
"""Trainium2 Bass kernel for nn_AggregationLayer (segment_reduce).

Strategy (8 NeuronCores, SPMD):
  - Shard the pixel axis: core j owns image rows [40j, 40j+40) of every image
    (F = 40*320 = 12800 pixels), for ALL 128 instances.
  - Masked sums (quat/scales/z + mask_size + per-class counts) as one PE
    matmul chain per core: lhsT = masksT [128px, 128inst] bf16 chunks,
    rhs = fieldsT [128px, 23ch x 8img] bf16 chunks, accumulated into a
    [128, 184] f32 PSUM tile. Fields are hi/lo bf16-split so the sums are
    accurate to ~1e-5 relative; binary masks are bf16-exact.
  - Per-instance image selection is applied AFTER the matmul with a
    host-built one-hot mask over the 8 image blocks (handles arbitrary
    sample_ids).
  - class_ids from per-class indicator counts (exact for binary masks).
  - xy[sample_ids] gather as a K=8 one-hot fp32 matmul (exact), then
    masks * xy on DVE in natural layout, streamed out.
  - Partial sums AllReduce'd across the 8 cores; finalize (means, quat
    normalize, exp, class) on device; core 0's outputs are used.

Falls back to a pure-numpy implementation when inputs are outside the
fast path's assumptions (non-binary masks, bad sample_ids, odd shapes).
"""

import os

import numpy as np
import ml_dtypes

import concourse.bass as bass
import concourse.bacc as bacc
import concourse.mybir as mybir
import concourse.tile as tile

B, H, W = 8, 320, 320
N = 128
NCORES = 8
ROWS = H // NCORES          # 40 rows of the image per core
F = ROWS * W                # 12800 pixels per core
PK = 128                    # pixels per matmul chunk
CH = F // PK                # 100 chunks
NCH = 23                    # sum channels per image (see layout below)
NSUM = NCH * B              # 184 matmul output columns
XGC = 1600                  # xy flat columns per 8-image group (16 groups)
XCH = 800                   # xy chunk columns (2 chunks per group)
NXC = 2 * F // XCH          # 32 xy chunks
XSL = 2560                  # masks slab size in flat elements
NCLS = 6

# channel layout (per image) for the sum matmul:
#   0:4   quat hi     4:7  scales hi   7    z hi
#   8:12  quat lo    12:15 scales lo  15    z lo
#   16    ones (mask_size)
#   17:23 class indicators (cat == 1..6)
BF16 = ml_dtypes.bfloat16

_CACHE = {}


def _build_bass():
    nc = bacc.Bacc("TRN2", target_bir_lowering=False, debug=False, num_devices=NCORES)
    dt = mybir.dt

    mT = nc.dram_tensor("mT", [F, N], dt.bfloat16, kind="ExternalInput")
    mN = nc.dram_tensor("mN", [N, F], dt.bfloat16, kind="ExternalInput")
    sfT = nc.dram_tensor("sfT", [F, NCH, B], dt.bfloat16, kind="ExternalInput")
    # xy hi/lo packed 16-groups-of-8-images on partitions: row j*8+b holds
    # group j's 1600 flat (ch, px) columns of image b.
    xyJh = nc.dram_tensor("xyJh", [N, XGC], dt.bfloat16, kind="ExternalInput")
    xyJl = nc.dram_tensor("xyJl", [N, XGC], dt.bfloat16, kind="ExternalInput")
    # one-hot selector variants: selv[64u + r*8 + b, r, n] = (sid[n] == b)
    selv = nc.dram_tensor("selv", [N, 8, N], dt.bfloat16, kind="ExternalInput")
    selm = nc.dram_tensor("selm", [N, NCH, B], dt.bfloat16, kind="ExternalInput")
    clsw = nc.dram_tensor("clsw", [N, NCLS], dt.float32, kind="ExternalInput")

    xyp = nc.dram_tensor("xyp", [N, 2 * F], dt.float32, kind="ExternalOutput")
    fin_o = nc.dram_tensor("fin", [N, 9], dt.float32, kind="ExternalOutput")

    mul = mybir.AluOpType.mult
    add = mybir.AluOpType.add
    AF = mybir.ActivationFunctionType

    with tile.TileContext(nc) as tc:
        with (
            tc.tile_pool(name="big", bufs=1) as big,
            tc.tile_pool(name="xyv_p", bufs=3) as xyv_p,
            tc.tile_pool(name="xout", bufs=6) as xout,
            tc.tile_pool(name="small", bufs=1) as small,
            tc.tile_pool(name="ps_s", bufs=1, space="PSUM") as ps_s,
            tc.tile_pool(name="ps_x", bufs=3, space="PSUM") as ps_x,
            tc.tile_pool(name="dram", bufs=1, space="DRAM") as dpool,
        ):
            # ---- small/constant loads
            selv_sb = small.tile([N, 8, N], dt.bfloat16)
            nc.gpsimd.dma_start(selv_sb, selv[:])
            selm_sb = small.tile([N, NCH, B], dt.bfloat16)
            nc.sync.dma_start(selm_sb, selm[:])
            clsw_sb = small.tile([N, NCLS], dt.float32)
            nc.sync.dma_start(clsw_sb, clsw[:])

            # ---- xy hi/lo (128-partition packed) + natural masks on the
            # SWDGE (gpsimd) ring so they don't contend with SP/ACT loads.
            xyJh_sb = xyv_p.tile([N, XGC], dt.bfloat16, tag="xyJh")
            nc.gpsimd.dma_start(xyJh_sb, xyJh[:])
            xyJl_sb = xyv_p.tile([N, XGC], dt.bfloat16, tag="xyJl")
            nc.gpsimd.dma_start(xyJl_sb, xyJl[:])
            # masks natural: 2 slabs early (first xy chunks), 3 after the
            # Pool-ring mT/sfT pair below.
            mN_sb = big.tile([N, F], dt.bfloat16, tag="mN")
            NMSL = F // XSL  # 5 range-writes into one tile
            for s in range(2):
                nc.gpsimd.dma_start(mN_sb[:, s * XSL:(s + 1) * XSL],
                                    mN[:, s * XSL:(s + 1) * XSL])

            # ---- big transposed loads: slab pairs 4 on SP, 4 on ACT, 2 on
            # the gpsimd ring, so the sums matmul chain is fed fast.
            NSLAB = 10
            SL = CH // NSLAB  # 10 chunks per slab
            mT_r = mT.rearrange("(c p) n -> p c n", p=PK)
            sfT_r = sfT.rearrange("(c p) ch b -> p c ch b", p=PK)
            mT_sl, sfT_sl = [], []
            for s in range(NSLAB):
                eng = (nc.sync, nc.scalar, nc.sync, nc.scalar, nc.gpsimd)[s % 5]
                t1 = big.tile([PK, SL, N], dt.bfloat16, tag=f"mT{s}")
                eng.dma_start(t1, mT_r[:, s * SL:(s + 1) * SL, :])
                mT_sl.append(t1)
                t2 = big.tile([PK, SL, NCH, B], dt.bfloat16, tag=f"sfT{s}")
                eng.dma_start(t2, sfT_r[:, s * SL:(s + 1) * SL, :, :])
                sfT_sl.append(t2)
                if s == 9:
                    for ms in range(2, NMSL):
                        nc.gpsimd.dma_start(
                            mN_sb[:, ms * XSL:(ms + 1) * XSL],
                            mN[:, ms * XSL:(ms + 1) * XSL])

            # ---- interleaved compute emission: sums-chain matmuls with xy
            # chunks woven in so the xy product/store stream starts early.
            def emit_xy(c):
                j, h = divmod(c, 2)
                base, r = 64 * (j // 8), j % 8
                ps = ps_x.tile([N, XCH], dt.float32, tag="psx")
                lhs = selv_sb[base:base + 64, r, :]
                for q0, q1 in ((0, 512), (512, XCH)):
                    cs = slice(h * XCH + q0, h * XCH + q1)
                    nc.tensor.matmul(ps[:, q0:q1], lhs,
                                     xyJh_sb[base:base + 64, cs],
                                     start=True, stop=False,
                                     skip_group_check=True)
                    nc.tensor.matmul(ps[:, q0:q1], lhs,
                                     xyJl_sb[base:base + 64, cs],
                                     start=False, stop=True,
                                     skip_group_check=True)
                px0 = (j % 8) * XGC + h * XCH
                ot = xout.tile([N, XCH], dt.float32, tag="xyout")
                nc.vector.tensor_tensor(ot, mN_sb[:, px0:px0 + XCH], ps, mul)
                st_eng = nc.sync if c % 2 == 0 else nc.scalar
                st_eng.dma_start(xyp[:, c * XCH:(c + 1) * XCH], ot)

            psum_s = ps_s.tile([N, NCH, B], dt.float32)
            xy_c = 0
            for k in range(CH):
                s, i = divmod(k, SL)
                nc.tensor.matmul(
                    psum_s, mT_sl[s][:, i, :], sfT_sl[s][:, i, :, :],
                    start=(k == 0), stop=(k == CH - 1),
                    skip_group_check=True,
                )
                if xy_c < NXC and k % 10 == 9:
                    emit_xy(xy_c)
                    xy_c += 1
            # select this instance's image block BEFORE the collective —
            # selection commutes with the cross-core sum and shrinks the
            # collective payload 8x (and runs off the critical tail).
            selp = small.tile([N, NCH, B], dt.float32)
            nc.vector.tensor_tensor(selp, psum_s, selm_sb, mul)
            pred = small.tile([N, NCH], dt.float32)
            nc.vector.tensor_reduce(pred, selp, axis=mybir.AxisListType.X, op=add)

            # ---- cross-core AllGather of the selected partial sums
            # (AllGather + on-device 8-way add is cheaper than AllReduce in
            # the collective engine: no reduce multiplier on the wire.)
            cc_in = dpool.tile([N, NCH], dt.float32)
            cc_out = dpool.tile([NCORES, N, NCH], dt.float32)
            nc.gpsimd.dma_start(cc_in[:], pred)
            nc.gpsimd.collective_compute(
                "AllGather",
                mybir.AluOpType.bypass,
                replica_groups=[list(range(NCORES))],
                ins=[cc_in.opt()],
                outs=[cc_out.opt()],
            )
            gath = small.tile([N, NCORES, NCH], dt.float32)
            nc.gpsimd.dma_start(gath, cc_out.rearrange("g n ch -> n g ch"))
            red4 = small.tile([N, 4, NCH], dt.float32)
            nc.vector.tensor_tensor(red4, gath[:, 0:4, :], gath[:, 4:8, :], add)
            red2 = small.tile([N, 2, NCH], dt.float32)
            nc.vector.tensor_tensor(red2, red4[:, 0:2, :], red4[:, 2:4, :], add)
            red = small.tile([N, NCH], dt.float32)
            nc.vector.tensor_tensor(red, red2[:, 0, :], red2[:, 1, :], add)

            # ---- finalize (tiny [128, *] ops) into one [N, 9] output:
            # [quat(4) | sc(3) | z_agg(1) | class(1)]
            fin = small.tile([N, 9], dt.float32)
            qsz = small.tile([N, 8], dt.float32)
            nc.vector.tensor_tensor(qsz, red[:, 0:8], red[:, 8:16], add)
            qm = small.tile([N, 4], dt.float32)
            inv = small.tile([N, 1], dt.float32)
            nc.vector.reciprocal(inv, red[:, 16:17])
            nc.vector.tensor_scalar(qm, qsz[:, 0:4], inv, None, mul)
            nc.vector.tensor_scalar(fin[:, 4:7], qsz[:, 4:7], inv, None, mul)
            nc.scalar.activation(fin[:, 7:8], qsz[:, 7:8], AF.Exp, scale=inv)

            # 1/sqrt(S) = Exp(-0.5 * Ln(S)): keeps every ACT op in the one
            # natural_log_exp table set (no mid-kernel table reload).
            sq = small.tile([N, 4], dt.float32)
            nc.vector.tensor_tensor(sq, qm, qm, mul)
            nrm2 = small.tile([N, 1], dt.float32)
            nc.vector.tensor_reduce(nrm2, sq, axis=mybir.AxisListType.X, op=add)
            nrmc = small.tile([N, 1], dt.float32)
            nc.vector.tensor_scalar(nrmc, nrm2, 1e-24, None, mybir.AluOpType.max)
            lnv = small.tile([N, 1], dt.float32)
            nc.scalar.activation(lnv, nrmc, AF.Ln)
            invn = small.tile([N, 1], dt.float32)
            nc.scalar.activation(invn, lnv, AF.Exp, scale=-0.5)
            nc.vector.tensor_scalar(fin[:, 0:4], qm, invn, None, mul)

            pos = small.tile([N, NCLS], dt.float32)
            nc.vector.tensor_scalar(pos, red[:, 17:23], 0.5, None,
                                    mybir.AluOpType.is_ge)
            wcls = small.tile([N, NCLS], dt.float32)
            nc.vector.tensor_tensor(wcls, pos, clsw_sb, mul)
            nc.vector.tensor_reduce(fin[:, 8:9], wcls, axis=mybir.AxisListType.X,
                                    op=mybir.AluOpType.max)

            nc.gpsimd.dma_start(fin_o[:], fin)

            # remaining xy chunks (emitted last so the collective/finalize
            # chain above outranks their stores in queue priority)
            while xy_c < NXC:
                emit_xy(xy_c)
                xy_c += 1

    nc.compile()
    return nc


def _get_nc():
    if "nc" not in _CACHE:
        _CACHE["nc"] = _build_bass()
    return _CACHE["nc"]


def _get_runner():
    """Build the jitted 8-core SPMD executable once and reuse it.

    Mirrors bass2jax.run_bass_via_pjrt's multi-core branch, but caches the
    jax.jit(shard_map(...)) so repeated kernel() calls do not recompile.
    """
    if "runner" in _CACHE:
        return _CACHE["runner"]
    import time as _time

    import jax
    from concourse import bass2jax
    from concourse import mybir as mb

    nc = _get_nc()
    bass2jax.install_neuronx_cc_hook()

    partition_name = (nc.partition_id_tensor.name
                      if nc.partition_id_tensor else None)
    in_names, out_names, out_avals, zero_shapes = [], [], [], []
    for alloc in nc.m.functions[0].allocations:
        if not isinstance(alloc, mb.MemoryLocationSet):
            continue
        name = alloc.memorylocations[0].name
        if alloc.kind == "ExternalInput":
            if name != partition_name:
                in_names.append(name)
        elif alloc.kind == "ExternalOutput":
            out_names.append(name)
            shape = tuple(alloc.tensor_shape)
            dtype = mb.dt.np(alloc.dtype)
            out_avals.append(jax.core.ShapedArray(shape, dtype))
            zero_shapes.append((shape, dtype))
    n_params = len(in_names)
    n_outs = len(out_avals)
    all_in_names = list(in_names) + list(out_names)
    if partition_name is not None:
        all_in_names.append(partition_name)
    donate = tuple(range(n_params, n_params + n_outs))

    def _body(*args):
        operands = list(args)
        if partition_name is not None:
            operands.append(bass2jax.partition_id_tensor())
        outs = bass2jax._bass_exec_p.bind(
            *operands,
            out_avals=tuple(out_avals),
            in_names=tuple(all_in_names),
            out_names=tuple(out_names),
            lowering_input_output_aliases=(),
            sim_require_finite=True,
            sim_require_nnan=True,
            nc=nc,
        )
        return tuple(outs)

    devices = jax.devices()[:NCORES]
    mesh = bass2jax.Mesh(np.asarray(devices), ("core",))
    in_specs = (bass2jax.PartitionSpec("core"),) * (n_params + n_outs)
    out_specs = (bass2jax.PartitionSpec("core"),) * len(out_names)
    sharded = jax.jit(
        bass2jax.shard_map(_body, mesh=mesh, in_specs=in_specs,
                           out_specs=out_specs, check_rep=False),
        donate_argnums=donate, keep_unused=True,
    )

    mesh_sharding = jax.sharding.NamedSharding(
        mesh, bass2jax.PartitionSpec("core"))

    def run(in_maps, fetch=True):
        per_core = [[np.asarray(m[name]) for name in in_names]
                    for m in in_maps]
        concat_in = [
            jax.device_put(
                np.concatenate([per_core[c][i] for c in range(NCORES)], axis=0),
                mesh_sharding)
            for i in range(n_params)
        ]
        concat_zeros = [
            jax.device_put(np.zeros((NCORES * s[0], *s[1:]), d), mesh_sharding)
            for s, d in zero_shapes
        ]
        jax.block_until_ready(concat_in)
        jax.block_until_ready(concat_zeros)
        t0 = _time.perf_counter()
        out_arrs = sharded(*concat_in, *concat_zeros)
        jax.block_until_ready(out_arrs)
        dt_s = _time.perf_counter() - t0
        if not fetch:
            return None, dt_s
        results = [
            {name: np.asarray(out_arrs[i]).reshape(NCORES, *out_avals[i].shape)[c]
             for i, name in enumerate(out_names)}
            for c in range(NCORES)
        ]
        return results, dt_s

    _CACHE["runner"] = run
    return run


def _numpy_ref(cat_mask, instance_masks, sample_ids, quaternion, scales, xy, z):
    masks = instance_masks.astype(np.float32)
    sid = sample_ids.astype(np.int64)
    mask_size = masks.sum(axis=(-2, -1))
    q = quaternion[sid]                       # [N,4,H,W]
    q_sum = np.einsum("nhw,nchw->nc", masks, q, optimize=True)
    quat = q_sum / mask_size[:, None]
    quat = quat / np.maximum(np.linalg.norm(quat, axis=1, keepdims=True), 1e-12)
    sc = np.einsum("nhw,nchw->nc", masks, scales[sid], optimize=True)
    sc = sc / mask_size[:, None]
    z_mean = np.einsum("nhw,nhw->n", masks, z[sid], optimize=True) / mask_size
    z_agg = np.exp(z_mean)[:, None].astype(np.float32)
    xy_masked = masks[:, None] * xy[sid]
    class_ids = np.max(
        masks * cat_mask[sid].astype(np.float32), axis=(-2, -1)
    ).astype(np.int32)
    return (class_ids, instance_masks, sample_ids,
            quat.astype(np.float32), sc.astype(np.float32),
            xy_masked.astype(np.float32), z_agg)


def _split_bf16(x):
    hi = x.astype(BF16)
    lo = (x - hi.astype(np.float32)).astype(BF16)
    return hi, lo


def kernel(cat_mask, instance_masks, sample_ids, quaternion, scales, xy, z):
    cat_mask = np.asarray(cat_mask)
    instance_masks = np.asarray(instance_masks, dtype=np.float32)
    sample_ids_in = np.asarray(sample_ids)
    quaternion = np.asarray(quaternion, dtype=np.float32)
    scales = np.asarray(scales, dtype=np.float32)
    xy = np.asarray(xy, dtype=np.float32)
    z = np.asarray(z, dtype=np.float32)
    sid = sample_ids_in.astype(np.int64)

    fast_ok = (
        instance_masks.shape == (N, H, W)
        and cat_mask.shape == (B, H, W)
        and quaternion.shape == (B, 4, H, W)
        and scales.shape == (B, 3, H, W)
        and xy.shape == (B, 2, H, W)
        and z.shape == (B, H, W)
        and sid.shape == (N,)
        and sid.min() >= 0 and sid.max() < B
        and np.all((instance_masks == 0.0) | (instance_masks == 1.0))
        and np.all((cat_mask >= 0) & (cat_mask <= NCLS))
    )
    if not fast_ok:
        return _numpy_ref(cat_mask, instance_masks, sample_ids_in,
                          quaternion, scales, xy, z)

    # ---- host-side shard preparation -------------------------------------
    qhi, qlo = _split_bf16(quaternion)          # [8,4,H,W]
    shi, slo = _split_bf16(scales)              # [8,3,H,W]
    zhi, zlo = _split_bf16(z)                   # [8,H,W]
    # sum-field stack [8, 23, H, W] in bf16
    sf = np.empty((B, NCH, H, W), dtype=BF16)
    sf[:, 0:4] = qhi
    sf[:, 4:7] = shi
    sf[:, 7] = zhi
    sf[:, 8:12] = qlo
    sf[:, 12:15] = slo
    sf[:, 15] = zlo
    sf[:, 16] = np.float32(1.0)
    for v in range(1, NCLS + 1):
        sf[:, 16 + v] = (cat_mask == v)

    xyh_np, xyl_np = _split_bf16(xy)                      # [8,2,H,W] bf16

    eq = (sid[:, None] == np.arange(B)[None, :])          # [N, 8]
    selm_np = np.ascontiguousarray(
        np.broadcast_to(eq[:, None, :], (N, NCH, B)).astype(BF16))
    clsw_np = np.ascontiguousarray(
        np.broadcast_to(np.arange(1, NCLS + 1, dtype=np.float32), (N, NCLS)))
    # selector variants for the K=64 grouped xy-gather matmul
    selv_np = np.zeros((N, 8, N), dtype=BF16)
    eqT = eq.T.astype(BF16)                               # [8, N]
    for r in range(8):
        for b in range(B):
            selv_np[r * 8 + b, r, :] = eqT[b]
            selv_np[64 + r * 8 + b, r, :] = eqT[b]

    def _pack_xy(a, rs):
        v = a[:, :, rs, :].reshape(B, 16, XGC)
        return np.ascontiguousarray(v.transpose(1, 0, 2).reshape(N, XGC))

    masks_bf = instance_masks.reshape(N, H, W).astype(BF16)
    in_maps = []
    for j in range(NCORES):
        rs = slice(ROWS * j, ROWS * (j + 1))
        m_slice = masks_bf[:, rs, :].reshape(N, F)
        in_maps.append({
            "mT": np.ascontiguousarray(m_slice.T),
            "mN": np.ascontiguousarray(m_slice),
            "sfT": np.ascontiguousarray(
                sf[:, :, rs, :].reshape(B, NCH, F).transpose(2, 1, 0)),
            "xyJh": _pack_xy(xyh_np, rs),
            "xyJl": _pack_xy(xyl_np, rs),
            "selv": selv_np,
            "selm": selm_np,
            "clsw": clsw_np,
        })

    run = _get_runner()
    res, dt_s = run(in_maps)
    kernel.last_wall_s = dt_s
    reps = int(os.environ.get("KERNEL_BENCH_REPS", "0"))
    if reps:
        times = []
        for _ in range(reps):
            _, dt_s = run(in_maps, fetch=False)
            times.append(dt_s)
        kernel.bench_times_s = times
        kernel.last_wall_s = min(times)

    xy_masked = np.concatenate(
        [res[j]["xyp"].reshape(N, 2, ROWS, W) for j in range(NCORES)], axis=2)
    fin = np.asarray(res[0]["fin"], dtype=np.float32)
    quat = np.ascontiguousarray(fin[:, 0:4])
    sc = np.ascontiguousarray(fin[:, 4:7])
    z_agg = np.ascontiguousarray(fin[:, 7:8])
    class_ids = fin[:, 8].astype(np.int32)

    return (class_ids, instance_masks, sample_ids_in, quat, sc,
            np.ascontiguousarray(xy_masked, dtype=np.float32), z_agg)


kernel.last_exec_time_ns = None
kernel.last_wall_s = None
kernel.bench_times_s = None


# revision 45
# speedup vs baseline: 1.0496x; 1.0496x over previous
"""Trainium2 Bass kernel for nn_AggregationLayer (segment_reduce).

Strategy (8 NeuronCores, SPMD):
  - Shard the pixel axis: core j owns image rows [40j, 40j+40) of every image
    (F = 40*320 = 12800 pixels), for ALL 128 instances.
  - Masked sums (quat/scales/z + mask_size + per-class counts) as one PE
    matmul chain per core: lhsT = masksT [128px, 128inst] bf16 chunks,
    rhs = fieldsT [128px, 23ch x 8img] bf16 chunks, accumulated into a
    [128, 184] f32 PSUM tile. Fields are hi/lo bf16-split so the sums are
    accurate to ~1e-5 relative; binary masks are bf16-exact.
  - Per-instance image selection is applied AFTER the matmul with a
    host-built one-hot mask over the 8 image blocks (handles arbitrary
    sample_ids).
  - class_ids from per-class indicator counts (exact for binary masks).
  - xy[sample_ids] gather as a K=8 one-hot fp32 matmul (exact), then
    masks * xy on DVE in natural layout, streamed out.
  - Partial sums AllReduce'd across the 8 cores; finalize (means, quat
    normalize, exp, class) on device; core 0's outputs are used.

Falls back to a pure-numpy implementation when inputs are outside the
fast path's assumptions (non-binary masks, bad sample_ids, odd shapes).
"""

import os

import numpy as np
import ml_dtypes

try:
    import jax as _jax
    _jax.config.update("jax_compilation_cache_dir", "/tmp/jax_neff_cache")
    _jax.config.update("jax_persistent_cache_min_compile_time_secs", 1.0)
except Exception:
    pass

import concourse.bass as bass
import concourse.bacc as bacc
import concourse.mybir as mybir
import concourse.tile as tile

B, H, W = 8, 320, 320
N = 128
NCORES = 8
ROWS = H // NCORES          # 40 rows of the image per core
F = ROWS * W                # 12800 pixels per core
PK = 128                    # pixels per matmul chunk
CH = F // PK                # 100 chunks
NCH = 23                    # sum channels per image (see layout below)
NSUM = NCH * B              # 184 matmul output columns
XGC = 1600                  # xy flat columns per 8-image group (16 groups)
XCH = 800                   # xy chunk columns (2 chunks per group)
NXC = 2 * F // XCH          # 32 xy chunks
XSL = 2560                  # masks slab size in flat elements
NCLS = 6

# channel layout (per image) for the sum matmul:
#   0:4   quat hi     4:7  scales hi   7    z hi
#   8:12  quat lo    12:15 scales lo  15    z lo
#   16    ones (mask_size)
#   17:23 class indicators (cat == 1..6)
BF16 = ml_dtypes.bfloat16

_CACHE = {}


def _build_bass():
    nc = bacc.Bacc("TRN2", target_bir_lowering=False, debug=False, num_devices=NCORES)
    dt = mybir.dt

    mT = nc.dram_tensor("mT", [F, N], dt.bfloat16, kind="ExternalInput")
    mN = nc.dram_tensor("mN", [N, F], dt.bfloat16, kind="ExternalInput")
    sfT = nc.dram_tensor("sfT", [F, NCH, B], dt.bfloat16, kind="ExternalInput")
    # xy hi/lo packed 16-groups-of-8-images on partitions: row j*8+b holds
    # group j's 1600 flat (ch, px) columns of image b.
    xyJh = nc.dram_tensor("xyJh", [N, XGC], dt.bfloat16, kind="ExternalInput")
    xyJl = nc.dram_tensor("xyJl", [N, XGC], dt.bfloat16, kind="ExternalInput")
    # one-hot selector variants: selv[64u + r*8 + b, r, n] = (sid[n] == b)
    selv = nc.dram_tensor("selv", [N, 8, N], dt.bfloat16, kind="ExternalInput")
    selm = nc.dram_tensor("selm", [N, NCH, B], dt.bfloat16, kind="ExternalInput")
    clsw = nc.dram_tensor("clsw", [N, NCLS], dt.float32, kind="ExternalInput")

    xyp = nc.dram_tensor("xyp", [N, 2 * F], dt.float32, kind="ExternalOutput")
    fin_o = nc.dram_tensor("fin", [N, 9], dt.float32, kind="ExternalOutput")

    mul = mybir.AluOpType.mult
    add = mybir.AluOpType.add
    AF = mybir.ActivationFunctionType

    with tile.TileContext(nc) as tc:
        with (
            tc.tile_pool(name="big", bufs=1) as big,
            tc.tile_pool(name="xyv_p", bufs=3) as xyv_p,
            tc.tile_pool(name="xout", bufs=6) as xout,
            tc.tile_pool(name="small", bufs=1) as small,
            tc.tile_pool(name="ps_s", bufs=1, space="PSUM") as ps_s,
            tc.tile_pool(name="ps_x", bufs=3, space="PSUM") as ps_x,
            tc.tile_pool(name="dram", bufs=1, space="DRAM") as dpool,
        ):
            # ---- small/constant loads
            selv_sb = small.tile([N, 8, N], dt.bfloat16)
            nc.gpsimd.dma_start(selv_sb, selv[:])
            selm_sb = small.tile([N, NCH, B], dt.bfloat16)
            nc.sync.dma_start(selm_sb, selm[:])
            clsw_sb = small.tile([N, NCLS], dt.float32)
            nc.sync.dma_start(clsw_sb, clsw[:])

            # ---- xy hi/lo (128-partition packed) + natural masks on the
            # SWDGE (gpsimd) ring so they don't contend with SP/ACT loads.
            xyJh_sb = xyv_p.tile([N, XGC], dt.bfloat16, tag="xyJh")
            nc.gpsimd.dma_start(xyJh_sb, xyJh[:])
            xyJl_sb = xyv_p.tile([N, XGC], dt.bfloat16, tag="xyJl")
            nc.gpsimd.dma_start(xyJl_sb, xyJl[:])
            # masks natural: 2 slabs early (first xy chunks), 3 after the
            # Pool-ring mT/sfT pair below.
            mN_sb = big.tile([N, F], dt.bfloat16, tag="mN")
            NMSL = F // XSL  # 5 range-writes into one tile
            for s in range(2):
                nc.gpsimd.dma_start(mN_sb[:, s * XSL:(s + 1) * XSL],
                                    mN[:, s * XSL:(s + 1) * XSL])

            # ---- big transposed loads: slab pairs 4 on SP, 4 on ACT, 2 on
            # the gpsimd ring, so the sums matmul chain is fed fast.
            NSLAB = 10
            SL = CH // NSLAB  # 10 chunks per slab
            mT_r = mT.rearrange("(c p) n -> p c n", p=PK)
            sfT_r = sfT.rearrange("(c p) ch b -> p c ch b", p=PK)
            mT_sl, sfT_sl = [], []
            for s in range(NSLAB):
                eng = (nc.sync, nc.scalar, nc.sync, nc.scalar, nc.gpsimd)[s % 5]
                t1 = big.tile([PK, SL, N], dt.bfloat16, tag=f"mT{s}")
                eng.dma_start(t1, mT_r[:, s * SL:(s + 1) * SL, :])
                mT_sl.append(t1)
                t2 = big.tile([PK, SL, NCH, B], dt.bfloat16, tag=f"sfT{s}")
                eng.dma_start(t2, sfT_r[:, s * SL:(s + 1) * SL, :, :])
                sfT_sl.append(t2)
                if s == 9:
                    for ms in range(2, NMSL):
                        nc.gpsimd.dma_start(
                            mN_sb[:, ms * XSL:(ms + 1) * XSL],
                            mN[:, ms * XSL:(ms + 1) * XSL])

            # ---- interleaved compute emission: sums-chain matmuls with xy
            # chunks woven in so the xy product/store stream starts early.
            def emit_xy(c):
                j, h = divmod(c, 2)
                base, r = 64 * (j // 8), j % 8
                ps = ps_x.tile([N, XCH], dt.float32, tag="psx")
                lhs = selv_sb[base:base + 64, r, :]
                for q0, q1 in ((0, 512), (512, XCH)):
                    cs = slice(h * XCH + q0, h * XCH + q1)
                    nc.tensor.matmul(ps[:, q0:q1], lhs,
                                     xyJh_sb[base:base + 64, cs],
                                     start=True, stop=False,
                                     skip_group_check=True)
                    nc.tensor.matmul(ps[:, q0:q1], lhs,
                                     xyJl_sb[base:base + 64, cs],
                                     start=False, stop=True,
                                     skip_group_check=True)
                px0 = (j % 8) * XGC + h * XCH
                ot = xout.tile([N, XCH], dt.float32, tag="xyout")
                nc.vector.tensor_tensor(ot, mN_sb[:, px0:px0 + XCH], ps, mul)
                st_eng = nc.sync if c % 2 == 0 else nc.scalar
                st_eng.dma_start(xyp[:, c * XCH:(c + 1) * XCH], ot)

            psum_s = ps_s.tile([N, NCH, B], dt.float32)
            xy_c = 0
            for k in range(CH):
                s, i = divmod(k, SL)
                nc.tensor.matmul(
                    psum_s, mT_sl[s][:, i, :], sfT_sl[s][:, i, :, :],
                    start=(k == 0), stop=(k == CH - 1),
                    skip_group_check=True,
                )
                if xy_c < NXC and k % 10 == 9:
                    emit_xy(xy_c)
                    xy_c += 1
            # select this instance's image block BEFORE the collective —
            # selection commutes with the cross-core sum and shrinks the
            # collective payload 8x (and runs off the critical tail).
            selp = small.tile([N, NCH, B], dt.float32)
            nc.vector.tensor_tensor(selp, psum_s, selm_sb, mul)
            pred = small.tile([N, NCH], dt.float32)
            nc.vector.tensor_reduce(pred, selp, axis=mybir.AxisListType.X, op=add)

            # ---- cross-core AllGather of the selected partial sums
            # (AllGather + on-device 8-way add is cheaper than AllReduce in
            # the collective engine: no reduce multiplier on the wire.)
            cc_in = dpool.tile([N, NCH], dt.float32)
            cc_out = dpool.tile([NCORES, N, NCH], dt.float32)
            nc.gpsimd.dma_start(cc_in[:], pred)
            nc.gpsimd.collective_compute(
                "AllGather",
                mybir.AluOpType.bypass,
                replica_groups=[list(range(NCORES))],
                ins=[cc_in.opt()],
                outs=[cc_out.opt()],
            )
            gath = small.tile([N, NCORES, NCH], dt.float32)
            nc.gpsimd.dma_start(gath, cc_out.rearrange("g n ch -> n g ch"))
            red4 = small.tile([N, 4, NCH], dt.float32)
            nc.vector.tensor_tensor(red4, gath[:, 0:4, :], gath[:, 4:8, :], add)
            red2 = small.tile([N, 2, NCH], dt.float32)
            nc.vector.tensor_tensor(red2, red4[:, 0:2, :], red4[:, 2:4, :], add)
            red = small.tile([N, NCH], dt.float32)
            nc.vector.tensor_tensor(red, red2[:, 0, :], red2[:, 1, :], add)

            # ---- finalize (tiny [128, *] ops) into one [N, 9] output:
            # [quat(4) | sc(3) | z_agg(1) | class(1)]
            fin = small.tile([N, 9], dt.float32)
            qsz = small.tile([N, 8], dt.float32)
            nc.vector.tensor_tensor(qsz, red[:, 0:8], red[:, 8:16], add)
            qm = small.tile([N, 4], dt.float32)
            inv = small.tile([N, 1], dt.float32)
            nc.vector.reciprocal(inv, red[:, 16:17])
            nc.vector.tensor_scalar(qm, qsz[:, 0:4], inv, None, mul)
            nc.vector.tensor_scalar(fin[:, 4:7], qsz[:, 4:7], inv, None, mul)
            nc.scalar.activation(fin[:, 7:8], qsz[:, 7:8], AF.Exp, scale=inv)

            # 1/sqrt(S) = Exp(-0.5 * Ln(S)): keeps every ACT op in the one
            # natural_log_exp table set (no mid-kernel table reload).
            sq = small.tile([N, 4], dt.float32)
            nc.vector.tensor_tensor(sq, qm, qm, mul)
            nrm2 = small.tile([N, 1], dt.float32)
            nc.vector.tensor_reduce(nrm2, sq, axis=mybir.AxisListType.X, op=add)
            nrmc = small.tile([N, 1], dt.float32)
            nc.vector.tensor_scalar(nrmc, nrm2, 1e-24, None, mybir.AluOpType.max)
            lnv = small.tile([N, 1], dt.float32)
            nc.scalar.activation(lnv, nrmc, AF.Ln)
            invn = small.tile([N, 1], dt.float32)
            nc.scalar.activation(invn, lnv, AF.Exp, scale=-0.5)
            nc.vector.tensor_scalar(fin[:, 0:4], qm, invn, None, mul)

            pos = small.tile([N, NCLS], dt.float32)
            nc.vector.tensor_scalar(pos, red[:, 17:23], 0.5, None,
                                    mybir.AluOpType.is_ge)
            wcls = small.tile([N, NCLS], dt.float32)
            nc.vector.tensor_tensor(wcls, pos, clsw_sb, mul)
            nc.vector.tensor_reduce(fin[:, 8:9], wcls, axis=mybir.AxisListType.X,
                                    op=mybir.AluOpType.max)

            nc.gpsimd.dma_start(fin_o[:], fin)

            # remaining xy chunks (emitted last so the collective/finalize
            # chain above outranks their stores in queue priority)
            while xy_c < NXC:
                emit_xy(xy_c)
                xy_c += 1

    nc.compile()
    return nc


def _get_nc():
    if "nc" not in _CACHE:
        _CACHE["nc"] = _build_bass()
    return _CACHE["nc"]


def _get_runner():
    """Build the jitted 8-core SPMD executable once and reuse it.

    Mirrors bass2jax.run_bass_via_pjrt's multi-core branch, but caches the
    jax.jit(shard_map(...)) so repeated kernel() calls do not recompile.
    """
    if "runner" in _CACHE:
        return _CACHE["runner"]
    import time as _time

    import jax
    from concourse import bass2jax
    from concourse import mybir as mb

    nc = _get_nc()
    bass2jax.install_neuronx_cc_hook()

    partition_name = (nc.partition_id_tensor.name
                      if nc.partition_id_tensor else None)
    in_names, out_names, out_avals, zero_shapes = [], [], [], []
    for alloc in nc.m.functions[0].allocations:
        if not isinstance(alloc, mb.MemoryLocationSet):
            continue
        name = alloc.memorylocations[0].name
        if alloc.kind == "ExternalInput":
            if name != partition_name:
                in_names.append(name)
        elif alloc.kind == "ExternalOutput":
            out_names.append(name)
            shape = tuple(alloc.tensor_shape)
            dtype = mb.dt.np(alloc.dtype)
            out_avals.append(jax.core.ShapedArray(shape, dtype))
            zero_shapes.append((shape, dtype))
    n_params = len(in_names)
    n_outs = len(out_avals)
    all_in_names = list(in_names) + list(out_names)
    if partition_name is not None:
        all_in_names.append(partition_name)
    donate = tuple(range(n_params, n_params + n_outs))

    def _body(*args):
        operands = list(args)
        if partition_name is not None:
            operands.append(bass2jax.partition_id_tensor())
        outs = bass2jax._bass_exec_p.bind(
            *operands,
            out_avals=tuple(out_avals),
            in_names=tuple(all_in_names),
            out_names=tuple(out_names),
            lowering_input_output_aliases=(),
            sim_require_finite=True,
            sim_require_nnan=True,
            nc=nc,
        )
        return tuple(outs)

    devices = jax.devices()[:NCORES]
    mesh = bass2jax.Mesh(np.asarray(devices), ("core",))
    in_specs = (bass2jax.PartitionSpec("core"),) * (n_params + n_outs)
    out_specs = (bass2jax.PartitionSpec("core"),) * len(out_names)
    sharded = jax.jit(
        bass2jax.shard_map(_body, mesh=mesh, in_specs=in_specs,
                           out_specs=out_specs, check_rep=False),
        donate_argnums=donate, keep_unused=True,
    )

    mesh_sharding = jax.sharding.NamedSharding(
        mesh, bass2jax.PartitionSpec("core"))

    def run(in_maps, fetch=True):
        per_core = [[np.asarray(m[name]) for name in in_names]
                    for m in in_maps]
        concat_in = [
            jax.device_put(
                np.concatenate([per_core[c][i] for c in range(NCORES)], axis=0),
                mesh_sharding)
            for i in range(n_params)
        ]
        concat_zeros = [
            jax.device_put(np.zeros((NCORES * s[0], *s[1:]), d), mesh_sharding)
            for s, d in zero_shapes
        ]
        jax.block_until_ready(concat_in)
        jax.block_until_ready(concat_zeros)
        t0 = _time.perf_counter()
        out_arrs = sharded(*concat_in, *concat_zeros)
        jax.block_until_ready(out_arrs)
        dt_s = _time.perf_counter() - t0
        if not fetch:
            return None, dt_s
        results = [
            {name: np.asarray(out_arrs[i]).reshape(NCORES, *out_avals[i].shape)[c]
             for i, name in enumerate(out_names)}
            for c in range(NCORES)
        ]
        return results, dt_s

    _CACHE["runner"] = run
    return run


def _numpy_ref(cat_mask, instance_masks, sample_ids, quaternion, scales, xy, z):
    masks = instance_masks.astype(np.float32)
    sid = sample_ids.astype(np.int64)
    mask_size = masks.sum(axis=(-2, -1))
    q = quaternion[sid]                       # [N,4,H,W]
    q_sum = np.einsum("nhw,nchw->nc", masks, q, optimize=True)
    quat = q_sum / mask_size[:, None]
    quat = quat / np.maximum(np.linalg.norm(quat, axis=1, keepdims=True), 1e-12)
    sc = np.einsum("nhw,nchw->nc", masks, scales[sid], optimize=True)
    sc = sc / mask_size[:, None]
    z_mean = np.einsum("nhw,nhw->n", masks, z[sid], optimize=True) / mask_size
    z_agg = np.exp(z_mean)[:, None].astype(np.float32)
    xy_masked = masks[:, None] * xy[sid]
    class_ids = np.max(
        masks * cat_mask[sid].astype(np.float32), axis=(-2, -1)
    ).astype(np.int32)
    return (class_ids, instance_masks, sample_ids,
            quat.astype(np.float32), sc.astype(np.float32),
            xy_masked.astype(np.float32), z_agg)


def _split_bf16(x):
    hi = x.astype(BF16)
    lo = (x - hi.astype(np.float32)).astype(BF16)
    return hi, lo


def kernel(cat_mask, instance_masks, sample_ids, quaternion, scales, xy, z):
    cat_mask = np.asarray(cat_mask)
    instance_masks = np.asarray(instance_masks, dtype=np.float32)
    sample_ids_in = np.asarray(sample_ids)
    quaternion = np.asarray(quaternion, dtype=np.float32)
    scales = np.asarray(scales, dtype=np.float32)
    xy = np.asarray(xy, dtype=np.float32)
    z = np.asarray(z, dtype=np.float32)
    sid = sample_ids_in.astype(np.int64)

    fast_ok = (
        instance_masks.shape == (N, H, W)
        and cat_mask.shape == (B, H, W)
        and quaternion.shape == (B, 4, H, W)
        and scales.shape == (B, 3, H, W)
        and xy.shape == (B, 2, H, W)
        and z.shape == (B, H, W)
        and sid.shape == (N,)
        and sid.min() >= 0 and sid.max() < B
        and np.all((instance_masks == 0.0) | (instance_masks == 1.0))
        and np.all((cat_mask >= 0) & (cat_mask <= NCLS))
    )
    if not fast_ok:
        return _numpy_ref(cat_mask, instance_masks, sample_ids_in,
                          quaternion, scales, xy, z)

    # ---- host-side shard preparation -------------------------------------
    qhi, qlo = _split_bf16(quaternion)          # [8,4,H,W]
    shi, slo = _split_bf16(scales)              # [8,3,H,W]
    zhi, zlo = _split_bf16(z)                   # [8,H,W]
    # sum-field stack [8, 23, H, W] in bf16
    sf = np.empty((B, NCH, H, W), dtype=BF16)
    sf[:, 0:4] = qhi
    sf[:, 4:7] = shi
    sf[:, 7] = zhi
    sf[:, 8:12] = qlo
    sf[:, 12:15] = slo
    sf[:, 15] = zlo
    sf[:, 16] = np.float32(1.0)
    for v in range(1, NCLS + 1):
        sf[:, 16 + v] = (cat_mask == v)

    xyh_np, xyl_np = _split_bf16(xy)                      # [8,2,H,W] bf16

    eq = (sid[:, None] == np.arange(B)[None, :])          # [N, 8]
    selm_np = np.ascontiguousarray(
        np.broadcast_to(eq[:, None, :], (N, NCH, B)).astype(BF16))
    clsw_np = np.ascontiguousarray(
        np.broadcast_to(np.arange(1, NCLS + 1, dtype=np.float32), (N, NCLS)))
    # selector variants for the K=64 grouped xy-gather matmul
    selv_np = np.zeros((N, 8, N), dtype=BF16)
    eqT = eq.T.astype(BF16)                               # [8, N]
    for r in range(8):
        for b in range(B):
            selv_np[r * 8 + b, r, :] = eqT[b]
            selv_np[64 + r * 8 + b, r, :] = eqT[b]

    def _pack_xy(a, rs):
        v = a[:, :, rs, :].reshape(B, 16, XGC)
        return np.ascontiguousarray(v.transpose(1, 0, 2).reshape(N, XGC))

    masks_bf = instance_masks.reshape(N, H, W).astype(BF16)
    in_maps = []
    for j in range(NCORES):
        rs = slice(ROWS * j, ROWS * (j + 1))
        m_slice = masks_bf[:, rs, :].reshape(N, F)
        in_maps.append({
            "mT": np.ascontiguousarray(m_slice.T),
            "mN": np.ascontiguousarray(m_slice),
            "sfT": np.ascontiguousarray(
                sf[:, :, rs, :].reshape(B, NCH, F).transpose(2, 1, 0)),
            "xyJh": _pack_xy(xyh_np, rs),
            "xyJl": _pack_xy(xyl_np, rs),
            "selv": selv_np,
            "selm": selm_np,
            "clsw": clsw_np,
        })

    run = _get_runner()
    res, dt_s = run(in_maps)
    kernel.last_wall_s = dt_s
    reps = int(os.environ.get("KERNEL_BENCH_REPS", "0"))
    if reps:
        times = []
        for _ in range(reps):
            _, dt_s = run(in_maps, fetch=False)
            times.append(dt_s)
        kernel.bench_times_s = times
        kernel.last_wall_s = min(times)

    xy_masked = np.concatenate(
        [res[j]["xyp"].reshape(N, 2, ROWS, W) for j in range(NCORES)], axis=2)
    fin = np.asarray(res[0]["fin"], dtype=np.float32)
    quat = np.ascontiguousarray(fin[:, 0:4])
    sc = np.ascontiguousarray(fin[:, 4:7])
    z_agg = np.ascontiguousarray(fin[:, 7:8])
    class_ids = fin[:, 8].astype(np.int32)

    return (class_ids, instance_masks, sample_ids_in, quat, sc,
            np.ascontiguousarray(xy_masked, dtype=np.float32), z_agg)


kernel.last_exec_time_ns = None
kernel.last_wall_s = None
kernel.bench_times_s = None
